# revision 10
# baseline (speedup 1.0000x reference)
"""Antisymmetric RNN kernel for Trainium2, data-parallel over batch on 8 cores.

Math (reference):
    M = W - W^T - gamma*I
    h_t = x_t @ V + bias                      [B, U]
    state_{t+1} = state_t + eps*tanh(h_t + state_t @ M)
    out[:, t] = state_{t+1}

Device formulation (per core, B_local=16):
    Rescale S' = state/eps, M' = eps*M  =>  S'_{t+1} = S'_t + tanh(z_t),
    z_t = h_t + S'_t @ M'. Keep everything transposed: partitions carry u
    (2 chunks of 128), free dim carries (chunk, batch) = 32 columns.

    The live value z_t = bias + x_t@V + S'_t@M' ping-pongs between two PSUM
    banks. Per step t (ACT reads bank p = t%2, PE/DVE prepare bank q):
      th_t = tanh(p)                 ScalarE, PSUM -> SBUF bf16 (chain)
      q := copy(p)                   DVE PSUM->PSUM, overlaps the tanh
      q += V@x_{t+1} - V@x_t         4 mms, overlap the tanh (telescopes
                                     exactly: both products round identically)
      q += M'[k,c] @ th_t[k]         4 mms, the only serial-chain PE work
    Dummy matmuls into a scratch bank keep the PE pipeline streaming so the
    chain matmuls issue warm (~32ns cadence instead of a ~200ns cold start).
    The output states S'_{t+1} = x0/eps + cumsum_t(th_t) are reconstructed
    off the critical path with DVE tensor_tensor_scan over the tanh history
    (one scan per (chunk, batch) column, spread 1-per-2-steps so they never
    block the per-step copy), then DMA'd out per 256-step chunk.
    Host multiplies by eps and re-layouts.
"""

import sys

sys.path.insert(0, "/opt/trn_rl_repo")

import numpy as np
import ml_dtypes

import concourse.bass as bass
import concourse.bacc as bacc
import concourse.mybir as mybir
import concourse.tile as tile

EPS = 0.01
GAMMA = 0.01
B, T, D, U = 128, 1024, 128, 256
NCORES = 8
BL = B // NCORES  # 16 batch rows per core
NK = U // 128  # 2 u-chunks
W32 = NK * BL  # 32 free columns = (chunk, batch)
CH = 256  # history chunk (timesteps) per scan/DMA-out block

F32 = mybir.dt.float32
BF16 = mybir.dt.bfloat16
BF16_NP = ml_dtypes.bfloat16

_CACHED = {}


def build_nc(t_steps=T):
    nc = bacc.Bacc(None, target_bir_lowering=False)
    x_d = nc.declare_dram_parameter("xT", [D, t_steps, BL], BF16, isOutput=False)
    m_d = nc.declare_dram_parameter("Mp", [128, NK, NK, 128], BF16, isOutput=False)
    v_d = nc.declare_dram_parameter("Vp", [D, 2, NK, 128], BF16, isOutput=False)
    b_d = nc.declare_dram_parameter("b2", [NK, 128], BF16, isOutput=False)
    s_d = nc.declare_dram_parameter("sel", [NK, W32], BF16, isOutput=False)
    xt_d = nc.declare_dram_parameter("x0t", [128, NK, BL], BF16, isOutput=False)
    xh_d = nc.declare_dram_parameter("x0h", [128, W32], F32, isOutput=False)
    o_d = nc.declare_dram_parameter("out", [128, t_steps, W32], F32, isOutput=True)

    Tanh = mybir.ActivationFunctionType.Tanh
    ADD = mybir.AluOpType.add
    BYPASS = mybir.AluOpType.bypass

    with tile.TileContext(nc) as tc:
        with (
            tc.tile_pool(name="const", bufs=1) as cpool,
            tc.tile_pool(name="xp", bufs=1) as xpool,
            tc.tile_pool(name="tb", bufs=1) as tbpool,
            tc.tile_pool(name="hist", bufs=2) as hpool,
            tc.tile_pool(name="ps", bufs=1, space=bass.MemorySpace.PSUM) as ppool,
        ):
            m_sb = cpool.tile([128, NK, NK, 128], BF16)
            v_sb = cpool.tile([D, 2, NK, 128], BF16)
            b_sb = cpool.tile([NK, 128], BF16)
            s_sb = cpool.tile([NK, W32], BF16)
            xt_sb = cpool.tile([128, NK, BL], BF16)
            xh_sb = cpool.tile([128, W32], F32)
            nc.sync.dma_start(m_sb[:], m_d[:])
            nc.sync.dma_start(v_sb[:], v_d[:])
            nc.sync.dma_start(b_sb[:], b_d[:])
            nc.sync.dma_start(s_sb[:], s_d[:])
            nc.sync.dma_start(xt_sb[:], xt_d[:])
            nc.sync.dma_start(xh_sb[:], xh_d[:])

            x_sb = xpool.tile([D, t_steps, BL], BF16)
            xch = 128 if t_steps % 128 == 0 else t_steps
            for i in range(t_steps // xch):
                sl = slice(i * xch, (i + 1) * xch)
                nc.sync.dma_start(x_sb[:, sl, :], x_d[:, sl, :])

            tb_sb = tbpool.tile([128, t_steps, W32], BF16)
            # Two mirrored PSUM accumulators (separate banks). ACT reads one
            # bank while PE applies catch-up updates to the other, so only
            # the 4 tanh-dependent M-matmuls sit on the serial chain.
            z_bank_a = ppool.tile([128, W32], F32, tag="zA")
            z_bank_b = ppool.tile([128, W32], F32, tag="zB")
            z_scratch = ppool.tile([128, BL], F32, tag="zScratch")
            z_banks = [z_bank_a, z_bank_b]

            def emit_xswap(zb, s):
                # h window swap: += V @ x_{s+1} - V @ x_s  (exactly telescopes)
                for c in range(NK):
                    zc = zb[:, c * BL : (c + 1) * BL]
                    nc.tensor.matmul(
                        zc, v_sb[:, 0, c, :], x_sb[:, s + 1, :], start=False, stop=False
                    )
                    nc.tensor.matmul(
                        zc, v_sb[:, 1, c, :], x_sb[:, s, :], start=False, stop=False
                    )

            def emit_m(zb, s, stop=False):
                # += M'[k,c] @ tanh_s[k]
                for c in range(NK):
                    zc = zb[:, c * BL : (c + 1) * BL]
                    for k in range(NK):
                        last = stop and c == NK - 1 and k == NK - 1
                        nc.tensor.matmul(
                            zc,
                            m_sb[:, k, c, :],
                            tb_sb[:, s, k * BL : (k + 1) * BL],
                            start=False,
                            stop=last,
                        )

            z_ps = z_banks[0]

            # ---- init both banks: Z_0 = bias + (x0/eps) @ M' + x_0 @ V ----
            for zb in z_banks:
                nc.tensor.matmul(zb[:], b_sb[:], s_sb[:], start=True, stop=False)
                for c in range(NK):
                    zc = zb[:, c * BL : (c + 1) * BL]
                    for k in range(NK):
                        nc.tensor.matmul(
                            zc, m_sb[:, k, c, :], xt_sb[:, k, :], start=False,
                            stop=False,
                        )
                    nc.tensor.matmul(
                        zc, v_sb[:, 0, c, :], x_sb[:, 0, :], start=False, stop=False
                    )

            # ---- recurrence ----
            # step t: ACT reads bank p = t%2 (holds Z_t). Concurrently DVE
            # copies p -> q (other bank); then PE applies x-swap_t (during
            # the tanh window, warmed by dummy matmuls into a scratch bank)
            # and the 4 tanh-dependent M-mms (the only serial-chain PE work).
            NDUM = 8
            prev_hist = None
            pending = []  # deferred DVE scan jobs, drained 1 per 2 steps

            def emit_scan(job):
                c0, ln, hist, ph, j, dma = job
                init = xh_sb[:, j : j + 1] if ph is None else ph[:, CH - 1, j : j + 1]
                nc.vector.tensor_tensor_scan(
                    hist[:, :ln, j],
                    tb_sb[:, c0 : c0 + ln, j],
                    tb_sb[:, c0 : c0 + ln, j],
                    init,
                    ADD,
                    BYPASS,
                )
                if dma:
                    nc.sync.dma_start(o_d[:, c0 : c0 + ln, :], hist[:, :ln, :])

            for t in range(t_steps):
                p = z_banks[t % 2]
                q = z_banks[(t + 1) % 2]
                nc.scalar.activation(tb_sb[:, t, :], p[:], Tanh)
                if t < t_steps - 1:
                    nc.vector.tensor_copy(q[:], p[:])
                    for _ in range(NDUM):
                        nc.tensor.matmul(
                            z_scratch[:],
                            v_sb[:, 0, 0, :],
                            x_sb[:, t, :],
                            start=True,
                            stop=True,
                        )
                    emit_xswap(q, t)
                    emit_m(q, t, stop=(t >= t_steps - 3))
                    if t % 2 == 0 and pending:
                        emit_scan(pending.pop(0))
                # chunk done: queue its 32 state-reconstruction scans
                if (t + 1) % CH == 0 or t == t_steps - 1:
                    ch_len = CH if (t + 1) % CH == 0 else (t + 1) % CH
                    c0 = t + 1 - ch_len
                    hist = hpool.tile([128, CH, W32], F32, tag="hist")
                    for j in range(W32):
                        pending.append((c0, ch_len, hist, prev_hist, j, j == W32 - 1))
                    prev_hist = hist
            for job in pending:
                emit_scan(job)

    nc.compile()
    return nc


def _prep_consts(V, W, bias, x0):
    M = W - W.T - GAMMA * np.eye(U, dtype=np.float32)
    Mp = (EPS * M).reshape(NK, 128, NK, 128).transpose(1, 0, 2, 3)
    Vr = V.reshape(D, NK, 128)
    Vp = np.stack([Vr, -Vr], axis=1)  # [D, 2, NK, 128]
    b2 = bias.reshape(NK, 128)
    sel = np.zeros((NK, W32), dtype=np.float32)
    for c in range(NK):
        sel[c, c * BL : (c + 1) * BL] = 1.0
    x0e = (x0 / EPS).astype(np.float32)
    x0t = np.broadcast_to(x0e.reshape(NK, 128).transpose(1, 0)[:, :, None], (128, NK, BL))
    x0h = np.ascontiguousarray(x0t).reshape(128, W32)
    return {
        "Mp": np.ascontiguousarray(Mp).astype(BF16_NP),
        "Vp": np.ascontiguousarray(Vp).astype(BF16_NP),
        "b2": np.ascontiguousarray(b2).astype(BF16_NP),
        "sel": np.ascontiguousarray(sel).astype(BF16_NP),
        "x0t": np.ascontiguousarray(x0t).astype(BF16_NP),
        "x0h": np.ascontiguousarray(x0h).astype(np.float32),
    }


def _install_ntff_hook():
    # Register the axon NTFF profile hook if the image's antenv lacks it,
    # so trace=True can return exec_time_ns. Harmless if anything fails.
    import types

    try:
        import antenv.axon_hooks  # noqa: F401

        return
    except ImportError:
        pass
    try:
        import antenv
        from trn_agent_boot.trn_boot import _ntff_profile_via_ctypes

        mod = types.ModuleType("antenv.axon_hooks")
        _h = [None]
        mod.set_axon_ntff_profile_hook = lambda h: _h.__setitem__(0, h)
        mod.get_axon_ntff_profile_hook = lambda: _h[0]
        sys.modules["antenv.axon_hooks"] = mod
        antenv.axon_hooks = mod
        mod.set_axon_ntff_profile_hook(
            _ntff_profile_via_ctypes("/opt/axon/libaxon_pjrt.so")
        )
    except Exception:
        pass


def kernel(inputs, V, W, bias, x0, _t_steps=None, _trace=False):
    _install_ntff_hook()
    from concourse.bass_utils import run_bass_kernel_spmd

    inputs = np.asarray(inputs, dtype=np.float32)
    V = np.asarray(V, dtype=np.float32)
    W = np.asarray(W, dtype=np.float32)
    bias = np.asarray(bias, dtype=np.float32)
    x0 = np.asarray(x0, dtype=np.float32)

    t_steps = _t_steps or inputs.shape[1]
    key = t_steps
    if key not in _CACHED:
        _CACHED[key] = build_nc(t_steps)
    nc = _CACHED[key]

    consts = _prep_consts(V, W, bias, x0)
    in_maps = []
    for i in range(NCORES):
        shard = inputs[i * BL : (i + 1) * BL, :t_steps, :]  # [16, t, 128]
        xT = np.ascontiguousarray(shard.transpose(2, 1, 0)).astype(BF16_NP)
        in_maps.append({"xT": xT, **consts})

    res = run_bass_kernel_spmd(
        nc, in_maps, list(range(NCORES)), trace=_trace
    )
    outs = []
    for i in range(NCORES):
        o = res.results[i]["out"]  # [128, t, 32] f32
        o = o.reshape(128, t_steps, NK, BL).transpose(3, 1, 2, 0).reshape(BL, t_steps, U)
        outs.append(o)
    full = np.concatenate(outs, axis=0) * EPS
    if _trace:
        return full.astype(np.float32), res
    return full.astype(np.float32)


# revision 16
# speedup vs baseline: 1.6836x; 1.6836x over previous
"""Antisymmetric RNN kernel for Trainium2, data-parallel over batch on 8 cores.

Math (reference):
    M = W - W^T - gamma*I
    h_t = x_t @ V + bias                      [B, U]
    state_{t+1} = state_t + eps*tanh(h_t + state_t @ M)
    out[:, t] = state_{t+1}

Device formulation (per core, B_local=16):
    Rescale S' = state/eps, M' = eps*M  =>  S'_{t+1} = S'_t + tanh(z_t),
    z_t = h_t + S'_t @ M'. Keep everything transposed: partitions carry u
    (2 chunks of 128), free dim carries (chunk, batch) = 32 columns.

    The live value z_t = bias + x_t@V + S'_t@M' is mirrored across two PSUM
    banks. Per step t, ACT reads bank p = t%2 (which holds Z_t); the other
    bank q (holding Z_{t-1}) is advanced to Z_{t+1} by PE matmuls:
      th_t = tanh(p)                ScalarE, PSUM -> SBUF bf16 (chain)
      q += upd_{t-1} catch-up       8 mms, ready at tanh start (overlap it)
      q += V@x_{t+1} - V@x_t        4 mms, overlap the tanh (telescopes
                                    exactly: both products round identically)
      q += 0@x (warmers)            keep the PE pipeline streaming so the
                                    chain matmuls issue warm (~32ns cadence
                                    instead of a ~200ns cold start)
      q += M'[k,c] @ th_t[k]        4 mms, the only serial-chain PE work
    The output states S'_{t+1} = x0/eps + cumsum_t(th_t) are reconstructed
    off the critical path with DVE tensor_tensor_scan over the tanh history
    (one scan per (chunk, batch) column, spread 1-per-2-steps), then DMA'd
    out per 256-step chunk. Host multiplies by eps and re-layouts.
"""

import sys

sys.path.insert(0, "/opt/trn_rl_repo")

import numpy as np
import ml_dtypes

import concourse.bass as bass
import concourse.bacc as bacc
import concourse.mybir as mybir
import concourse.tile as tile

EPS = 0.01
GAMMA = 0.01
B, T, D, U = 128, 1024, 128, 256
NCORES = 8
BL = B // NCORES  # 16 batch rows per core
NK = U // 128  # 2 u-chunks
W32 = NK * BL  # 32 free columns = (chunk, batch)
CH = 256  # history chunk (timesteps) per scan/DMA-out block

F32 = mybir.dt.float32
BF16 = mybir.dt.bfloat16
BF16_NP = ml_dtypes.bfloat16

_CACHED = {}


def build_nc(t_steps=T):
    nc = bacc.Bacc(None, target_bir_lowering=False)
    x_d = nc.declare_dram_parameter("xT", [D, t_steps, BL], BF16, isOutput=False)
    m_d = nc.declare_dram_parameter("Mp", [128, NK, NK, 128], BF16, isOutput=False)
    v_d = nc.declare_dram_parameter("Vp", [D, 2, NK, 128], BF16, isOutput=False)
    b_d = nc.declare_dram_parameter("b2", [NK, 128], BF16, isOutput=False)
    s_d = nc.declare_dram_parameter("sel", [NK, W32], BF16, isOutput=False)
    xt_d = nc.declare_dram_parameter("x0t", [128, NK, BL], BF16, isOutput=False)
    xh_d = nc.declare_dram_parameter("x0h", [128, W32], F32, isOutput=False)
    zw_d = nc.declare_dram_parameter("zw", [128, 128], BF16, isOutput=False)
    o_d = nc.declare_dram_parameter("out", [128, t_steps, W32], F32, isOutput=True)

    Tanh = mybir.ActivationFunctionType.Tanh
    ADD = mybir.AluOpType.add
    BYPASS = mybir.AluOpType.bypass

    with tile.TileContext(nc) as tc:
        with (
            tc.tile_pool(name="const", bufs=1) as cpool,
            tc.tile_pool(name="xp", bufs=1) as xpool,
            tc.tile_pool(name="tb", bufs=1) as tbpool,
            tc.tile_pool(name="hist", bufs=2) as hpool,
            tc.tile_pool(name="ps", bufs=1, space=bass.MemorySpace.PSUM) as ppool,
        ):
            m_sb = cpool.tile([128, NK, NK, 128], BF16)
            v_sb = cpool.tile([D, 2, NK, 128], BF16)
            b_sb = cpool.tile([NK, 128], BF16)
            s_sb = cpool.tile([NK, W32], BF16)
            xt_sb = cpool.tile([128, NK, BL], BF16)
            xh_sb = cpool.tile([128, W32], F32)
            zw_sb = cpool.tile([128, 128], BF16)
            nc.sync.dma_start(zw_sb[:], zw_d[:])
            nc.sync.dma_start(m_sb[:], m_d[:])
            nc.sync.dma_start(v_sb[:], v_d[:])
            nc.sync.dma_start(b_sb[:], b_d[:])
            nc.sync.dma_start(s_sb[:], s_d[:])
            nc.sync.dma_start(xt_sb[:], xt_d[:])
            nc.sync.dma_start(xh_sb[:], xh_d[:])

            x_sb = xpool.tile([D, t_steps, BL], BF16)
            xch = 128 if t_steps % 128 == 0 else t_steps
            for i in range(t_steps // xch):
                sl = slice(i * xch, (i + 1) * xch)
                nc.sync.dma_start(x_sb[:, sl, :], x_d[:, sl, :])

            tb_sb = tbpool.tile([128, t_steps, W32], BF16)
            # Two mirrored PSUM accumulators (separate banks). ACT reads one
            # bank while PE applies catch-up updates to the other, so only
            # the 4 tanh-dependent M-matmuls sit on the serial chain.
            z_bank_a = ppool.tile([128, W32], F32, tag="zA")
            z_bank_b = ppool.tile([128, W32], F32, tag="zB")
            z_banks = [z_bank_a, z_bank_b]

            def emit_xswap(zb, s):
                # h window swap: += V @ x_{s+1} - V @ x_s  (exactly telescopes)
                for c in range(NK):
                    zc = zb[:, c * BL : (c + 1) * BL]
                    nc.tensor.matmul(
                        zc, v_sb[:, 0, c, :], x_sb[:, s + 1, :], start=False, stop=False
                    )
                    nc.tensor.matmul(
                        zc, v_sb[:, 1, c, :], x_sb[:, s, :], start=False, stop=False
                    )

            def emit_m(zb, s, stop=False):
                # += M'[k,c] @ tanh_s[k]
                for c in range(NK):
                    zc = zb[:, c * BL : (c + 1) * BL]
                    for k in range(NK):
                        last = stop and c == NK - 1 and k == NK - 1
                        nc.tensor.matmul(
                            zc,
                            m_sb[:, k, c, :],
                            tb_sb[:, s, k * BL : (k + 1) * BL],
                            start=False,
                            stop=last,
                        )

            z_ps = z_banks[0]

            # ---- init both banks: Z_0 = bias + (x0/eps) @ M' + x_0 @ V ----
            for zb in z_banks:
                nc.tensor.matmul(zb[:], b_sb[:], s_sb[:], start=True, stop=False)
                for c in range(NK):
                    zc = zb[:, c * BL : (c + 1) * BL]
                    for k in range(NK):
                        nc.tensor.matmul(
                            zc, m_sb[:, k, c, :], xt_sb[:, k, :], start=False,
                            stop=False,
                        )
                    nc.tensor.matmul(
                        zc, v_sb[:, 0, c, :], x_sb[:, 0, :], start=False, stop=False
                    )

            # ---- recurrence ----
            # step t: ACT reads bank p = t%2 (holds Z_t). Concurrently DVE
            # copies p -> q (other bank); then PE applies x-swap_t (during
            # the tanh window, warmed by dummy matmuls into a scratch bank)
            # and the 4 tanh-dependent M-mms (the only serial-chain PE work).
            NDUM = 5
            prev_hist = None
            pending = []  # deferred DVE scan jobs, drained 1 per 2 steps

            def emit_scan(job):
                c0, ln, hist, ph, j, dma = job
                init = xh_sb[:, j : j + 1] if ph is None else ph[:, CH - 1, j : j + 1]
                nc.vector.tensor_tensor_scan(
                    hist[:, :ln, j],
                    tb_sb[:, c0 : c0 + ln, j],
                    tb_sb[:, c0 : c0 + ln, j],
                    init,
                    ADD,
                    BYPASS,
                )
                if dma:
                    nc.sync.dma_start(o_d[:, c0 : c0 + ln, :], hist[:, :ln, :])

            for t in range(t_steps):
                p = z_banks[t % 2]
                q = z_banks[(t + 1) % 2]
                nc.scalar.activation(tb_sb[:, t, :], p[:], Tanh)
                if t < t_steps - 1:
                    # catch-up: re-apply upd_{t-1} to q (ready at tanh start,
                    # overlaps it), then this step's x-swap, then zero-weight
                    # warmers (exact no-ops on q) that keep the PE pipeline
                    # streaming until the chain M-mms issue, so those go at
                    # ~32ns cadence instead of a ~200ns cold start.
                    if t >= 1:
                        emit_xswap(q, t - 1)
                        emit_m(q, t - 1, stop=False)
                    emit_xswap(q, t)
                    for _ in range(NDUM):
                        nc.tensor.matmul(
                            q[:, :BL], zw_sb[:], x_sb[:, t, :], start=False, stop=False
                        )
                    emit_m(q, t, stop=(t >= t_steps - 3))
                    if t % 2 == 0 and pending:
                        emit_scan(pending.pop(0))
                # chunk done: queue its 32 state-reconstruction scans
                if (t + 1) % CH == 0 or t == t_steps - 1:
                    ch_len = CH if (t + 1) % CH == 0 else (t + 1) % CH
                    c0 = t + 1 - ch_len
                    hist = hpool.tile([128, CH, W32], F32, tag="hist")
                    for j in range(W32):
                        pending.append((c0, ch_len, hist, prev_hist, j, j == W32 - 1))
                    prev_hist = hist
            for job in pending:
                emit_scan(job)

    nc.compile()
    return nc


def _prep_consts(V, W, bias, x0):
    M = W - W.T - GAMMA * np.eye(U, dtype=np.float32)
    Mp = (EPS * M).reshape(NK, 128, NK, 128).transpose(1, 0, 2, 3)
    Vr = V.reshape(D, NK, 128)
    Vp = np.stack([Vr, -Vr], axis=1)  # [D, 2, NK, 128]
    b2 = bias.reshape(NK, 128)
    sel = np.zeros((NK, W32), dtype=np.float32)
    for c in range(NK):
        sel[c, c * BL : (c + 1) * BL] = 1.0
    x0e = (x0 / EPS).astype(np.float32)
    x0t = np.broadcast_to(x0e.reshape(NK, 128).transpose(1, 0)[:, :, None], (128, NK, BL))
    x0h = np.ascontiguousarray(x0t).reshape(128, W32)
    return {
        "Mp": np.ascontiguousarray(Mp).astype(BF16_NP),
        "Vp": np.ascontiguousarray(Vp).astype(BF16_NP),
        "b2": np.ascontiguousarray(b2).astype(BF16_NP),
        "sel": np.ascontiguousarray(sel).astype(BF16_NP),
        "x0t": np.ascontiguousarray(x0t).astype(BF16_NP),
        "x0h": np.ascontiguousarray(x0h).astype(np.float32),
        "zw": np.zeros((128, 128), dtype=BF16_NP),
    }


def _install_ntff_hook():
    # Register the axon NTFF profile hook if the image's antenv lacks it,
    # so trace=True can return exec_time_ns. Harmless if anything fails.
    import types

    try:
        import antenv.axon_hooks  # noqa: F401

        return
    except ImportError:
        pass
    try:
        import antenv
        from trn_agent_boot.trn_boot import _ntff_profile_via_ctypes

        mod = types.ModuleType("antenv.axon_hooks")
        _h = [None]
        mod.set_axon_ntff_profile_hook = lambda h: _h.__setitem__(0, h)
        mod.get_axon_ntff_profile_hook = lambda: _h[0]
        sys.modules["antenv.axon_hooks"] = mod
        antenv.axon_hooks = mod
        mod.set_axon_ntff_profile_hook(
            _ntff_profile_via_ctypes("/opt/axon/libaxon_pjrt.so")
        )
    except Exception:
        pass


def kernel(inputs, V, W, bias, x0, _t_steps=None, _trace=False):
    _install_ntff_hook()
    from concourse.bass_utils import run_bass_kernel_spmd

    inputs = np.asarray(inputs, dtype=np.float32)
    V = np.asarray(V, dtype=np.float32)
    W = np.asarray(W, dtype=np.float32)
    bias = np.asarray(bias, dtype=np.float32)
    x0 = np.asarray(x0, dtype=np.float32)

    t_steps = _t_steps or inputs.shape[1]
    key = t_steps
    if key not in _CACHED:
        _CACHED[key] = build_nc(t_steps)
    nc = _CACHED[key]

    consts = _prep_consts(V, W, bias, x0)
    in_maps = []
    for i in range(NCORES):
        shard = inputs[i * BL : (i + 1) * BL, :t_steps, :]  # [16, t, 128]
        xT = np.ascontiguousarray(shard.transpose(2, 1, 0)).astype(BF16_NP)
        in_maps.append({"xT": xT, **consts})

    res = run_bass_kernel_spmd(
        nc, in_maps, list(range(NCORES)), trace=_trace
    )
    outs = []
    for i in range(NCORES):
        o = res.results[i]["out"]  # [128, t, 32] f32
        o = o.reshape(128, t_steps, NK, BL).transpose(3, 1, 2, 0).reshape(BL, t_steps, U)
        outs.append(o)
    full = np.concatenate(outs, axis=0) * EPS
    if _trace:
        return full.astype(np.float32), res
    return full.astype(np.float32)


# revision 18
# speedup vs baseline: 2.1206x; 1.2596x over previous
"""Antisymmetric RNN kernel for Trainium2, data-parallel over batch on 8 cores.

Math (reference):
    M = W - W^T - gamma*I
    h_t = x_t @ V + bias                      [B, U]
    state_{t+1} = state_t + eps*tanh(h_t + state_t @ M)
    out[:, t] = state_{t+1}

Device formulation (per core, B_local=16):
    Rescale S' = state/eps, M' = eps*M  =>  S'_{t+1} = S'_t + tanh(z_t),
    z_t = h_t + S'_t @ M'. Keep everything transposed: partitions carry u
    (2 chunks of 128), free dim carries (chunk, batch) = 32 columns.

    The live value z_t = bias + x_t@V + S'_t@M' is mirrored across two PSUM
    banks. Per step t, ACT reads bank p = t%2 (which holds Z_t); the other
    bank q (holding Z_{t-1}) is advanced to Z_{t+1} by PE matmuls:
      th_t = tanh(p)                ScalarE, PSUM -> SBUF bf16 (chain)
      q += upd_{t-1} catch-up       8 mms, ready at tanh start (overlap it)
      q += V@x_{t+1} - V@x_t        4 mms, overlap the tanh (telescopes
                                    exactly: both products round identically)
      q += 0@x (warmers)            keep the PE pipeline streaming so the
                                    chain matmuls issue warm (~32ns cadence
                                    instead of a ~200ns cold start)
      q += M'[k,c] @ th_t[k]        4 mms, the only serial-chain PE work
    The output states S'_{t+1} = x0/eps + cumsum_t(th_t) are reconstructed
    off the critical path with DVE tensor_tensor_scan over the tanh history
    (one scan per (chunk, batch) column, spread 1-per-2-steps), then DMA'd
    out per 256-step chunk. Host multiplies by eps and re-layouts.
"""

import sys

sys.path.insert(0, "/opt/trn_rl_repo")

import numpy as np
import ml_dtypes

import concourse.bass as bass
import concourse.bacc as bacc
import concourse.mybir as mybir
import concourse.tile as tile

EPS = 0.01
GAMMA = 0.01
B, T, D, U = 128, 1024, 128, 256
NCORES = 8
BL = B // NCORES  # 16 batch rows per core
NK = U // 128  # 2 u-chunks
W32 = NK * BL  # 32 free columns = (chunk, batch)
CH = 256  # history chunk (timesteps) per scan/DMA-out block

F32 = mybir.dt.float32
BF16 = mybir.dt.bfloat16
BF16_NP = ml_dtypes.bfloat16

_CACHED = {}


def build_nc(t_steps=T):
    nc = bacc.Bacc(None, target_bir_lowering=False)
    x_d = nc.declare_dram_parameter("xT", [D, t_steps, BL], BF16, isOutput=False)
    m_d = nc.declare_dram_parameter("Mp", [128, NK, NK, 128], BF16, isOutput=False)
    v_d = nc.declare_dram_parameter("Vp", [D, 2, NK, 128], BF16, isOutput=False)
    b_d = nc.declare_dram_parameter("b2", [NK, 128], BF16, isOutput=False)
    s_d = nc.declare_dram_parameter("sel", [NK, W32], BF16, isOutput=False)
    xt_d = nc.declare_dram_parameter("x0t", [128, NK, BL], BF16, isOutput=False)
    xh_d = nc.declare_dram_parameter("x0h", [128, W32], F32, isOutput=False)
    zw_d = nc.declare_dram_parameter("zw", [128, 128], BF16, isOutput=False)
    o_d = nc.declare_dram_parameter("out", [128, t_steps, W32], F32, isOutput=True)

    Tanh = mybir.ActivationFunctionType.Tanh
    ADD = mybir.AluOpType.add
    BYPASS = mybir.AluOpType.bypass

    with tile.TileContext(nc) as tc:
        with (
            tc.tile_pool(name="const", bufs=1) as cpool,
            tc.tile_pool(name="xp", bufs=1) as xpool,
            tc.tile_pool(name="tb", bufs=1) as tbpool,
            tc.tile_pool(name="hist", bufs=2) as hpool,
            tc.tile_pool(name="ps", bufs=1, space=bass.MemorySpace.PSUM) as ppool,
        ):
            m_sb = cpool.tile([128, NK, NK, 128], BF16)
            v_sb = cpool.tile([D, 2, NK, 128], BF16)
            b_sb = cpool.tile([NK, 128], BF16)
            s_sb = cpool.tile([NK, W32], BF16)
            xt_sb = cpool.tile([128, NK, BL], BF16)
            xh_sb = cpool.tile([128, W32], F32)
            zw_sb = cpool.tile([128, 128], BF16)
            nc.sync.dma_start(zw_sb[:], zw_d[:])
            nc.sync.dma_start(m_sb[:], m_d[:])
            nc.sync.dma_start(v_sb[:], v_d[:])
            nc.sync.dma_start(b_sb[:], b_d[:])
            nc.sync.dma_start(s_sb[:], s_d[:])
            nc.sync.dma_start(xt_sb[:], xt_d[:])
            nc.sync.dma_start(xh_sb[:], xh_d[:])

            x_sb = xpool.tile([D, t_steps, BL], BF16)
            xch = 128 if t_steps % 128 == 0 else t_steps
            for i in range(t_steps // xch):
                sl = slice(i * xch, (i + 1) * xch)
                nc.sync.dma_start(x_sb[:, sl, :], x_d[:, sl, :])

            tb_sb = tbpool.tile([128, t_steps, W32], BF16)
            # Two mirrored PSUM accumulators (separate banks). ACT reads one
            # bank while PE applies catch-up updates to the other, so only
            # the 4 tanh-dependent M-matmuls sit on the serial chain.
            z_bank_a = ppool.tile([128, W32], F32, tag="zA")
            z_bank_b = ppool.tile([128, W32], F32, tag="zB")
            z_banks = [z_bank_a, z_bank_b]

            def emit_xswap(zb, s):
                # h window swap: += V @ x_{s+1} - V @ x_s  (exactly telescopes)
                for c in range(NK):
                    zc = zb[:, c * BL : (c + 1) * BL]
                    nc.tensor.matmul(
                        zc, v_sb[:, 0, c, :], x_sb[:, s + 1, :], start=False, stop=False
                    )
                    nc.tensor.matmul(
                        zc, v_sb[:, 1, c, :], x_sb[:, s, :], start=False, stop=False
                    )

            def emit_m(zb, s, stop=False):
                # += M'[k,c] @ tanh_s[k]
                for c in range(NK):
                    zc = zb[:, c * BL : (c + 1) * BL]
                    for k in range(NK):
                        last = stop and c == NK - 1 and k == NK - 1
                        nc.tensor.matmul(
                            zc,
                            m_sb[:, k, c, :],
                            tb_sb[:, s, k * BL : (k + 1) * BL],
                            start=False,
                            stop=last,
                        )

            z_ps = z_banks[0]

            # ---- init both banks: Z_0 = bias + (x0/eps) @ M' + x_0 @ V ----
            for zb in z_banks:
                nc.tensor.matmul(zb[:], b_sb[:], s_sb[:], start=True, stop=False)
                for c in range(NK):
                    zc = zb[:, c * BL : (c + 1) * BL]
                    for k in range(NK):
                        nc.tensor.matmul(
                            zc, m_sb[:, k, c, :], xt_sb[:, k, :], start=False,
                            stop=False,
                        )
                    nc.tensor.matmul(
                        zc, v_sb[:, 0, c, :], x_sb[:, 0, :], start=False, stop=False
                    )

            # ---- recurrence ----
            # step t: ACT reads bank p = t%2 (holds Z_t). Concurrently DVE
            # copies p -> q (other bank); then PE applies x-swap_t (during
            # the tanh window, warmed by dummy matmuls into a scratch bank)
            # and the 4 tanh-dependent M-mms (the only serial-chain PE work).
            prev_hist = None
            prev_len = CH
            pending = []  # deferred DVE scan jobs, drained 1 per 2 steps

            # chunk boundaries: 256-chunks, with the tail split smaller so the
            # final post-loop scan drain is short
            bounds = []
            pos = 0
            while pos < t_steps:
                rem = t_steps - pos
                if rem > CH:
                    step_len = CH
                elif rem > 128 and pos + rem == t_steps:
                    step_len = rem - 128
                elif rem > 64:
                    step_len = rem - 64
                else:
                    step_len = rem
                pos += step_len
                bounds.append(pos)
            bset = set(bounds)

            def emit_scan(job):
                c0, ln, hist, ph, pl, j, dma = job
                init = xh_sb[:, j : j + 1] if ph is None else ph[:, pl - 1, j : j + 1]
                nc.vector.tensor_tensor_scan(
                    hist[:, :ln, j],
                    tb_sb[:, c0 : c0 + ln, j],
                    tb_sb[:, c0 : c0 + ln, j],
                    init,
                    ADD,
                    BYPASS,
                )
                if dma:
                    nc.sync.dma_start(o_d[:, c0 : c0 + ln, :], hist[:, :ln, :])

            for t in range(t_steps):
                p = z_banks[t % 2]
                q = z_banks[(t + 1) % 2]
                nc.scalar.activation(tb_sb[:, t, :], p[:], Tanh)
                if t < t_steps - 1:
                    # catch-up: re-apply upd_{t-1} to q (ready at tanh start,
                    # overlaps it), then this step's x-swap, then zero-weight
                    # warmers (exact no-ops on q) that keep the PE pipeline
                    # streaming until the chain M-mms issue, so those go at
                    # ~32ns cadence instead of a ~200ns cold start.
                    if t >= 1:
                        emit_xswap(q, t - 1)
                        emit_m(q, t - 1, stop=False)
                    emit_xswap(q, t)
                    emit_m(q, t, stop=(t >= t_steps - 3))
                    if t % 2 == 0 and pending:
                        emit_scan(pending.pop(0))
                # chunk done: queue its 32 state-reconstruction scans
                if (t + 1) in bset:
                    c0 = 0 if (t + 1) == bounds[0] else bounds[bounds.index(t + 1) - 1]
                    ch_len = t + 1 - c0
                    hist = hpool.tile([128, CH, W32], F32, tag="hist")
                    for j in range(W32):
                        pending.append(
                            (c0, ch_len, hist, prev_hist, prev_len, j, j == W32 - 1)
                        )
                    prev_hist = hist
                    prev_len = ch_len
            for job in pending:
                emit_scan(job)

    nc.compile()
    return nc


def _prep_consts(V, W, bias, x0):
    M = W - W.T - GAMMA * np.eye(U, dtype=np.float32)
    Mp = (EPS * M).reshape(NK, 128, NK, 128).transpose(1, 0, 2, 3)
    Vr = V.reshape(D, NK, 128)
    Vp = np.stack([Vr, -Vr], axis=1)  # [D, 2, NK, 128]
    b2 = bias.reshape(NK, 128)
    sel = np.zeros((NK, W32), dtype=np.float32)
    for c in range(NK):
        sel[c, c * BL : (c + 1) * BL] = 1.0
    x0e = (x0 / EPS).astype(np.float32)
    x0t = np.broadcast_to(x0e.reshape(NK, 128).transpose(1, 0)[:, :, None], (128, NK, BL))
    x0h = np.ascontiguousarray(x0t).reshape(128, W32)
    return {
        "Mp": np.ascontiguousarray(Mp).astype(BF16_NP),
        "Vp": np.ascontiguousarray(Vp).astype(BF16_NP),
        "b2": np.ascontiguousarray(b2).astype(BF16_NP),
        "sel": np.ascontiguousarray(sel).astype(BF16_NP),
        "x0t": np.ascontiguousarray(x0t).astype(BF16_NP),
        "x0h": np.ascontiguousarray(x0h).astype(np.float32),
        "zw": np.zeros((128, 128), dtype=BF16_NP),
    }


def _install_ntff_hook():
    # Register the axon NTFF profile hook if the image's antenv lacks it,
    # so trace=True can return exec_time_ns. Harmless if anything fails.
    import types

    try:
        import antenv.axon_hooks  # noqa: F401

        return
    except ImportError:
        pass
    try:
        import antenv
        from trn_agent_boot.trn_boot import _ntff_profile_via_ctypes

        mod = types.ModuleType("antenv.axon_hooks")
        _h = [None]
        mod.set_axon_ntff_profile_hook = lambda h: _h.__setitem__(0, h)
        mod.get_axon_ntff_profile_hook = lambda: _h[0]
        sys.modules["antenv.axon_hooks"] = mod
        antenv.axon_hooks = mod
        mod.set_axon_ntff_profile_hook(
            _ntff_profile_via_ctypes("/opt/axon/libaxon_pjrt.so")
        )
    except Exception:
        pass


def kernel(inputs, V, W, bias, x0, _t_steps=None, _trace=False):
    _install_ntff_hook()
    from concourse.bass_utils import run_bass_kernel_spmd

    inputs = np.asarray(inputs, dtype=np.float32)
    V = np.asarray(V, dtype=np.float32)
    W = np.asarray(W, dtype=np.float32)
    bias = np.asarray(bias, dtype=np.float32)
    x0 = np.asarray(x0, dtype=np.float32)

    t_steps = _t_steps or inputs.shape[1]
    key = t_steps
    if key not in _CACHED:
        _CACHED[key] = build_nc(t_steps)
    nc = _CACHED[key]

    consts = _prep_consts(V, W, bias, x0)
    in_maps = []
    for i in range(NCORES):
        shard = inputs[i * BL : (i + 1) * BL, :t_steps, :]  # [16, t, 128]
        xT = np.ascontiguousarray(shard.transpose(2, 1, 0)).astype(BF16_NP)
        in_maps.append({"xT": xT, **consts})

    res = run_bass_kernel_spmd(
        nc, in_maps, list(range(NCORES)), trace=_trace
    )
    outs = []
    for i in range(NCORES):
        o = res.results[i]["out"]  # [128, t, 32] f32
        o = o.reshape(128, t_steps, NK, BL).transpose(3, 1, 2, 0).reshape(BL, t_steps, U)
        outs.append(o)
    full = np.concatenate(outs, axis=0) * EPS
    if _trace:
        return full.astype(np.float32), res
    return full.astype(np.float32)


# revision 19
# speedup vs baseline: 2.1227x; 1.0010x over previous
"""Antisymmetric RNN kernel for Trainium2, data-parallel over batch on 8 cores.

Math (reference):
    M = W - W^T - gamma*I
    h_t = x_t @ V + bias                      [B, U]
    state_{t+1} = state_t + eps*tanh(h_t + state_t @ M)
    out[:, t] = state_{t+1}

Device formulation (per core, B_local=16):
    Rescale S' = state/eps, M' = eps*M  =>  S'_{t+1} = S'_t + tanh(z_t),
    z_t = h_t + S'_t @ M'. Keep everything transposed: partitions carry u
    (2 chunks of 128), free dim carries (chunk, batch) = 32 columns.

    The live value z_t = bias + x_t@V + S'_t@M' is mirrored across two PSUM
    banks. Per step t, ACT reads bank p = t%2 (which holds Z_t); the other
    bank q (holding Z_{t-1}) is advanced to Z_{t+1} by PE matmuls:
      th_t = tanh(p)                ScalarE, PSUM -> SBUF bf16 (chain)
      q += upd_{t-1} catch-up       8 mms, ready at tanh start (overlap it)
      q += V@x_{t+1} - V@x_t        4 mms, overlap the tanh (telescopes
                                    exactly: both products round identically)
      q += 0@x (warmers)            keep the PE pipeline streaming so the
                                    chain matmuls issue warm (~32ns cadence
                                    instead of a ~200ns cold start)
      q += M'[k,c] @ th_t[k]        4 mms, the only serial-chain PE work
    The output states S'_{t+1} = x0/eps + cumsum_t(th_t) are reconstructed
    off the critical path with DVE tensor_tensor_scan over the tanh history
    (one scan per (chunk, batch) column, spread 1-per-2-steps), then DMA'd
    out per 256-step chunk. Host multiplies by eps and re-layouts.
"""

import sys

sys.path.insert(0, "/opt/trn_rl_repo")

import numpy as np
import ml_dtypes

import concourse.bass as bass
import concourse.bacc as bacc
import concourse.mybir as mybir
import concourse.tile as tile

EPS = 0.01
GAMMA = 0.01
B, T, D, U = 128, 1024, 128, 256
NCORES = 8
BL = B // NCORES  # 16 batch rows per core
NK = U // 128  # 2 u-chunks
W32 = NK * BL  # 32 free columns = (chunk, batch)
CH = 256  # history chunk (timesteps) per scan/DMA-out block

F32 = mybir.dt.float32
BF16 = mybir.dt.bfloat16
BF16_NP = ml_dtypes.bfloat16

_CACHED = {}


def build_nc(t_steps=T):
    nc = bacc.Bacc(None, target_bir_lowering=False)
    x_d = nc.declare_dram_parameter("xT", [D, t_steps, BL], BF16, isOutput=False)
    m_d = nc.declare_dram_parameter("Mp", [128, NK, NK, 128], BF16, isOutput=False)
    v_d = nc.declare_dram_parameter("Vp", [D, 2, NK, 128], BF16, isOutput=False)
    b_d = nc.declare_dram_parameter("b2", [NK, 128], BF16, isOutput=False)
    s_d = nc.declare_dram_parameter("sel", [NK, W32], BF16, isOutput=False)
    xt_d = nc.declare_dram_parameter("x0t", [128, NK, BL], BF16, isOutput=False)
    xh_d = nc.declare_dram_parameter("x0h", [128, W32], F32, isOutput=False)
    zw_d = nc.declare_dram_parameter("zw", [128, 128], BF16, isOutput=False)
    o_d = nc.declare_dram_parameter("out", [128, t_steps, W32], F32, isOutput=True)

    Tanh = mybir.ActivationFunctionType.Tanh
    ADD = mybir.AluOpType.add
    BYPASS = mybir.AluOpType.bypass

    with tile.TileContext(nc) as tc:
        with (
            tc.tile_pool(name="const", bufs=1) as cpool,
            tc.tile_pool(name="xp", bufs=1) as xpool,
            tc.tile_pool(name="tb", bufs=1) as tbpool,
            tc.tile_pool(name="hist", bufs=2) as hpool,
            tc.tile_pool(name="ps", bufs=1, space=bass.MemorySpace.PSUM) as ppool,
        ):
            m_sb = cpool.tile([128, NK, NK, 128], BF16)
            v_sb = cpool.tile([D, 2, NK, 128], BF16)
            b_sb = cpool.tile([NK, 128], BF16)
            s_sb = cpool.tile([NK, W32], BF16)
            xt_sb = cpool.tile([128, NK, BL], BF16)
            xh_sb = cpool.tile([128, W32], F32)
            zw_sb = cpool.tile([128, 128], BF16)
            nc.sync.dma_start(zw_sb[:], zw_d[:])
            nc.sync.dma_start(m_sb[:], m_d[:])
            nc.sync.dma_start(v_sb[:], v_d[:])
            nc.sync.dma_start(b_sb[:], b_d[:])
            nc.sync.dma_start(s_sb[:], s_d[:])
            nc.sync.dma_start(xt_sb[:], xt_d[:])
            nc.sync.dma_start(xh_sb[:], xh_d[:])

            x_sb = xpool.tile([D, t_steps, BL], BF16)
            xch = 128 if t_steps % 128 == 0 else t_steps
            for i in range(t_steps // xch):
                sl = slice(i * xch, (i + 1) * xch)
                nc.sync.dma_start(x_sb[:, sl, :], x_d[:, sl, :])

            tb_sb = tbpool.tile([128, t_steps, W32], BF16)
            # Two mirrored PSUM accumulators (separate banks). ACT reads one
            # bank while PE applies catch-up updates to the other, so only
            # the 4 tanh-dependent M-matmuls sit on the serial chain.
            z_bank_a = ppool.tile([128, W32], F32, tag="zA")
            z_bank_b = ppool.tile([128, W32], F32, tag="zB")
            z_banks = [z_bank_a, z_bank_b]

            def emit_xswap(zb, s):
                # h window swap: += V @ x_{s+1} - V @ x_s  (exactly telescopes)
                for c in range(NK):
                    zc = zb[:, c * BL : (c + 1) * BL]
                    nc.tensor.matmul(
                        zc, v_sb[:, 0, c, :], x_sb[:, s + 1, :], start=False, stop=False
                    )
                    nc.tensor.matmul(
                        zc, v_sb[:, 1, c, :], x_sb[:, s, :], start=False, stop=False
                    )

            def emit_m(zb, s, stop=False):
                # += M'[k,c] @ tanh_s[k]
                for c in range(NK):
                    zc = zb[:, c * BL : (c + 1) * BL]
                    for k in range(NK):
                        last = stop and c == NK - 1 and k == NK - 1
                        nc.tensor.matmul(
                            zc,
                            m_sb[:, k, c, :],
                            tb_sb[:, s, k * BL : (k + 1) * BL],
                            start=False,
                            stop=last,
                        )

            z_ps = z_banks[0]

            # ---- init both banks: Z_0 = bias + (x0/eps) @ M' + x_0 @ V ----
            for zb in z_banks:
                nc.tensor.matmul(zb[:], b_sb[:], s_sb[:], start=True, stop=False)
                for c in range(NK):
                    zc = zb[:, c * BL : (c + 1) * BL]
                    for k in range(NK):
                        nc.tensor.matmul(
                            zc, m_sb[:, k, c, :], xt_sb[:, k, :], start=False,
                            stop=False,
                        )
                    nc.tensor.matmul(
                        zc, v_sb[:, 0, c, :], x_sb[:, 0, :], start=False, stop=False
                    )

            # ---- recurrence ----
            # step t: ACT reads bank p = t%2 (holds Z_t). Concurrently DVE
            # copies p -> q (other bank); then PE applies x-swap_t (during
            # the tanh window, warmed by dummy matmuls into a scratch bank)
            # and the 4 tanh-dependent M-mms (the only serial-chain PE work).
            prev_hist = None
            prev_len = CH
            pending = []  # deferred DVE scan jobs, drained 1 per 2 steps

            # chunk boundaries: 256-chunks, with the tail split smaller so the
            # final post-loop scan drain is short
            bounds = []
            pos = 0
            while pos < t_steps:
                rem = t_steps - pos
                if rem > CH:
                    step_len = CH
                elif rem > 128 and pos + rem == t_steps:
                    step_len = rem - 128
                elif rem > 64:
                    step_len = rem - 64
                else:
                    step_len = rem
                pos += step_len
                bounds.append(pos)
            bset = set(bounds)

            def emit_scan(job):
                c0, ln, hist, ph, pl, j, dma = job
                init = xh_sb[:, j : j + 1] if ph is None else ph[:, pl - 1, j : j + 1]
                nc.vector.tensor_tensor_scan(
                    hist[:, :ln, j],
                    tb_sb[:, c0 : c0 + ln, j],
                    tb_sb[:, c0 : c0 + ln, j],
                    init,
                    ADD,
                    BYPASS,
                )
                if dma:
                    nc.sync.dma_start(o_d[:, c0 : c0 + ln, :], hist[:, :ln, :])

            for t in range(t_steps):
                p = z_banks[t % 2]
                q = z_banks[(t + 1) % 2]
                nc.scalar.activation(tb_sb[:, t, :], p[:], Tanh)
                if t < t_steps - 1:
                    # catch-up (ready at tanh start, overlaps it): bank q holds
                    # Z_{t-1}; advance its h-window two steps in one go --
                    # xswap_{t-1} + xswap_t telescopes to +V@x_{t+1} - V@x_{t-1}
                    # (the +-V@x_t pair cancels exactly), then re-apply M_{t-1}.
                    # Only M-mms_t are on the serial chain.
                    if t >= 1:
                        for c in range(NK):
                            zc = q[:, c * BL : (c + 1) * BL]
                            nc.tensor.matmul(
                                zc,
                                v_sb[:, 0, c, :],
                                x_sb[:, t + 1, :],
                                start=False,
                                stop=False,
                            )
                            nc.tensor.matmul(
                                zc,
                                v_sb[:, 1, c, :],
                                x_sb[:, t - 1, :],
                                start=False,
                                stop=False,
                            )
                        emit_m(q, t - 1, stop=False)
                    else:
                        emit_xswap(q, t)
                    emit_m(q, t, stop=(t >= t_steps - 3))
                    if t % 2 == 0 and pending:
                        emit_scan(pending.pop(0))
                # chunk done: queue its 32 state-reconstruction scans
                if (t + 1) in bset:
                    c0 = 0 if (t + 1) == bounds[0] else bounds[bounds.index(t + 1) - 1]
                    ch_len = t + 1 - c0
                    hist = hpool.tile([128, CH, W32], F32, tag="hist")
                    for j in range(W32):
                        pending.append(
                            (c0, ch_len, hist, prev_hist, prev_len, j, j == W32 - 1)
                        )
                    prev_hist = hist
                    prev_len = ch_len
            for job in pending:
                emit_scan(job)

    nc.compile()
    return nc


def _prep_consts(V, W, bias, x0):
    M = W - W.T - GAMMA * np.eye(U, dtype=np.float32)
    Mp = (EPS * M).reshape(NK, 128, NK, 128).transpose(1, 0, 2, 3)
    Vr = V.reshape(D, NK, 128)
    Vp = np.stack([Vr, -Vr], axis=1)  # [D, 2, NK, 128]
    b2 = bias.reshape(NK, 128)
    sel = np.zeros((NK, W32), dtype=np.float32)
    for c in range(NK):
        sel[c, c * BL : (c + 1) * BL] = 1.0
    x0e = (x0 / EPS).astype(np.float32)
    x0t = np.broadcast_to(x0e.reshape(NK, 128).transpose(1, 0)[:, :, None], (128, NK, BL))
    x0h = np.ascontiguousarray(x0t).reshape(128, W32)
    return {
        "Mp": np.ascontiguousarray(Mp).astype(BF16_NP),
        "Vp": np.ascontiguousarray(Vp).astype(BF16_NP),
        "b2": np.ascontiguousarray(b2).astype(BF16_NP),
        "sel": np.ascontiguousarray(sel).astype(BF16_NP),
        "x0t": np.ascontiguousarray(x0t).astype(BF16_NP),
        "x0h": np.ascontiguousarray(x0h).astype(np.float32),
        "zw": np.zeros((128, 128), dtype=BF16_NP),
    }


def _install_ntff_hook():
    # Register the axon NTFF profile hook if the image's antenv lacks it,
    # so trace=True can return exec_time_ns. Harmless if anything fails.
    import types

    try:
        import antenv.axon_hooks  # noqa: F401

        return
    except ImportError:
        pass
    try:
        import antenv
        from trn_agent_boot.trn_boot import _ntff_profile_via_ctypes

        mod = types.ModuleType("antenv.axon_hooks")
        _h = [None]
        mod.set_axon_ntff_profile_hook = lambda h: _h.__setitem__(0, h)
        mod.get_axon_ntff_profile_hook = lambda: _h[0]
        sys.modules["antenv.axon_hooks"] = mod
        antenv.axon_hooks = mod
        mod.set_axon_ntff_profile_hook(
            _ntff_profile_via_ctypes("/opt/axon/libaxon_pjrt.so")
        )
    except Exception:
        pass


def kernel(inputs, V, W, bias, x0, _t_steps=None, _trace=False):
    _install_ntff_hook()
    from concourse.bass_utils import run_bass_kernel_spmd

    inputs = np.asarray(inputs, dtype=np.float32)
    V = np.asarray(V, dtype=np.float32)
    W = np.asarray(W, dtype=np.float32)
    bias = np.asarray(bias, dtype=np.float32)
    x0 = np.asarray(x0, dtype=np.float32)

    t_steps = _t_steps or inputs.shape[1]
    key = t_steps
    if key not in _CACHED:
        _CACHED[key] = build_nc(t_steps)
    nc = _CACHED[key]

    consts = _prep_consts(V, W, bias, x0)
    in_maps = []
    for i in range(NCORES):
        shard = inputs[i * BL : (i + 1) * BL, :t_steps, :]  # [16, t, 128]
        xT = np.ascontiguousarray(shard.transpose(2, 1, 0)).astype(BF16_NP)
        in_maps.append({"xT": xT, **consts})

    res = run_bass_kernel_spmd(
        nc, in_maps, list(range(NCORES)), trace=_trace
    )
    outs = []
    for i in range(NCORES):
        o = res.results[i]["out"]  # [128, t, 32] f32
        o = o.reshape(128, t_steps, NK, BL).transpose(3, 1, 2, 0).reshape(BL, t_steps, U)
        outs.append(o)
    full = np.concatenate(outs, axis=0) * EPS
    if _trace:
        return full.astype(np.float32), res
    return full.astype(np.float32)
